# revision 10
# baseline (speedup 1.0000x reference)
"""GroupQueryAttention Trainium2 kernel (8 NeuronCores, SPMD).

Sharding: core i -> batch b = i//4, KV-head-pair hp = i%4.
Each core owns 2 KV heads (global 2*hp, 2*hp+1) and their 8 query heads
(global 8*hp .. 8*hp+7) for one batch element.

Per-core pipeline (all fp32 data, matmuls in float32r):
  phase A: q^T/k^T/v^T projections from host-pre-transposed X^T inputs
           (weights stationary, X^T moving; bias added on PSUM evac)
  phase B: per (local q-head, q-block of 256):
           scores^T = k^T.T @ q^T  (keys on partitions)
           exp fused into PSUM->SBUF evac (ACT)
           ctx^T accumulated with lhsT=[v|ones] -> softmax denominators free
           attn tiles PE-transposed back to (q, keys), normalized by 1/denom
           on evac (per-partition scalar), streamed to DRAM
  phase C: ctx re-transposed, partial out-projection out_p = ctx @ Wo_rows
Host: pre-transposes X, pre-scales Wq by D^-0.5, sums the 4 partial outs
per batch, adds bo, and reassembles attn head slices.
"""

import numpy as np
from contextlib import ExitStack

import concourse.bass as bass
import concourse.mybir as mybir
import concourse.tile as tile
from concourse import bacc
from concourse.bass_utils import run_bass_kernel_spmd
from concourse.masks import make_identity

FP = mybir.dt.float32
FPR = mybir.dt.float32r
AF = mybir.ActivationFunctionType

E = 2048          # embed dim
S = 2048          # tokens per batch (per core)
D = 64            # head dim
LQH = 8           # local q heads per core
QCOLS = LQH * D   # 512 q-proj cols per core
KVCOLS = 2 * D    # 128 k/v-proj cols per core (2 KV heads)
EC = E // 128     # 16 contraction chunks
NKC = S // 128    # 16 key chunks
QB = 256          # q block (free dim of scores^T matmuls)
NQB = S // QB     # 8 q blocks per head
TB = 256          # token block in phase A
NTB = S // TB     # 8


def _r(ap):
    return ap.bitcast(FPR)


def build_nc():
    nc = bacc.Bacc("TRN2", target_bir_lowering=False, debug=False, num_devices=8)

    xqT = nc.declare_dram_parameter("xqT", [E, S], FP, isOutput=False)
    xkT = nc.declare_dram_parameter("xkT", [E, S], FP, isOutput=False)
    xvT = nc.declare_dram_parameter("xvT", [E, S], FP, isOutput=False)
    wq = nc.declare_dram_parameter("wq", [E, QCOLS], FP, isOutput=False)
    wk = nc.declare_dram_parameter("wk", [E, KVCOLS], FP, isOutput=False)
    wv = nc.declare_dram_parameter("wv", [E, KVCOLS], FP, isOutput=False)
    wo = nc.declare_dram_parameter("wo", [QCOLS, E], FP, isOutput=False)
    bq = nc.declare_dram_parameter("bq", [128, 4], FP, isOutput=False)
    bkv = nc.declare_dram_parameter("bkv", [128, 2], FP, isOutput=False)
    attn = nc.declare_dram_parameter("attn", [LQH, S, S], FP, isOutput=True)
    outp = nc.declare_dram_parameter("outp", [S, E], FP, isOutput=True)

    with tile.TileContext(nc) as tc:
        with ExitStack() as ctx:
            _body(ctx, tc, xqT, xkT, xvT, wq, wk, wv, wo, bq, bkv, attn, outp)
    nc.compile()
    return nc


def _body(ctx, tc, xqT, xkT, xvT, wq, wk, wv, wo, bq, bkv, attn, outp):
    nc = tc.nc

    const_pool = ctx.enter_context(tc.tile_pool(name="const", bufs=1))
    identity = const_pool.tile([128, 128], FP)
    make_identity(nc, identity[:])
    biasq = const_pool.tile([128, 4], FP)
    nc.sync.dma_start(biasq[:], bq[:, :])
    biaskv = const_pool.tile([128, 2], FP)
    nc.sync.dma_start(biaskv[:], bkv[:, :])
    # reciprocal softmax denominators: col (qh*16 + qtile)
    recip = const_pool.tile([128, LQH * NKC], FP)

    # ---- persistent activations (live through phase C) ----
    pp = ctx.enter_context(tc.tile_pool(name="persist", bufs=1))
    qT = pp.tile([128, 4 * S], FP)       # 4 m-tiles of 128 d-rows x S tokens
    kT = pp.tile([128, S], FP)           # rows 0-63 kv-head 0, 64-127 kv-head 1
    vT = pp.tile([128, S], FP)
    vaug = pp.tile([128, 2 * NKC * 65], FP)  # per (h,kc): [128 keys, 64 v + 1 ones]

    # ================= phase A: projections =================
    with tc.tile_pool(name="wres", bufs=1) as wpool, \
         tc.tile_pool(name="xstream", bufs=4) as xpool, \
         tc.tile_pool(name="proj_psum", bufs=4, space="PSUM") as ppsum:
        wq_sb = wpool.tile([128, EC * QCOLS], FP)
        nc.sync.dma_start(
            _r(wq_sb[:].rearrange("p (c d) -> p c d", d=QCOLS)),
            _r(wq[:, :].rearrange("(c p) d -> p c d", p=128)),
        )
        wk_sb = wpool.tile([128, EC * KVCOLS], FP)
        nc.sync.dma_start(
            _r(wk_sb[:].rearrange("p (c d) -> p c d", d=KVCOLS)),
            _r(wk[:, :].rearrange("(c p) d -> p c d", p=128)),
        )
        wv_sb = wpool.tile([128, EC * KVCOLS], FP)
        nc.sync.dma_start(
            _r(wv_sb[:].rearrange("p (c d) -> p c d", d=KVCOLS)),
            _r(wv[:, :].rearrange("(c p) d -> p c d", p=128)),
        )

        for tb in range(NTB):
            t0 = tb * TB
            xq_blk = xpool.tile([128, EC * TB], FP, tag="xblk")
            nc.sync.dma_start(
                _r(xq_blk[:].rearrange("p (c t) -> p c t", t=TB)),
                _r(xqT[:, t0:t0 + TB].rearrange("(c p) t -> p c t", p=128)),
            )
            xk_blk = xpool.tile([128, EC * TB], FP, tag="xblk")
            nc.sync.dma_start(
                _r(xk_blk[:].rearrange("p (c t) -> p c t", t=TB)),
                _r(xkT[:, t0:t0 + TB].rearrange("(c p) t -> p c t", p=128)),
            )
            xv_blk = xpool.tile([128, EC * TB], FP, tag="xblk")
            nc.sync.dma_start(
                _r(xv_blk[:].rearrange("p (c t) -> p c t", t=TB)),
                _r(xvT[:, t0:t0 + TB].rearrange("(c p) t -> p c t", p=128)),
            )

            # m 0..3: q m-tiles; m 4: k; m 5: v
            for m in range(6):
                ps = ppsum.tile([128, TB], FP, tag="pp")
                if m < 4:
                    xb, wsb, wcols, col0 = xq_blk, wq_sb, QCOLS, m * 128
                elif m == 4:
                    xb, wsb, wcols, col0 = xk_blk, wk_sb, KVCOLS, 0
                else:
                    xb, wsb, wcols, col0 = xv_blk, wv_sb, KVCOLS, 0
                for ec in range(EC):
                    nc.tensor.matmul(
                        ps[:],
                        lhsT=_r(wsb[:, ec * wcols + col0: ec * wcols + col0 + 128]),
                        rhs=_r(xb[:, ec * TB:(ec + 1) * TB]),
                        start=(ec == 0),
                        stop=(ec == EC - 1),
                    )
                if m < 4:
                    dst = qT[:, m * S + t0: m * S + t0 + TB]
                    bias = biasq[:, m:m + 1]
                elif m == 4:
                    dst = kT[:, t0:t0 + TB]
                    bias = biaskv[:, 0:1]
                else:
                    dst = vT[:, t0:t0 + TB]
                    bias = biaskv[:, 1:2]
                nc.scalar.activation(_r(dst), ps[:], AF.Identity, bias=bias, scale=1.0)

        # build v_aug: transpose vT per head, append ones column
        with tc.tile_pool(name="vtp", bufs=4, space="PSUM") as vtpool:
            ones_t = const_pool.tile([128, 2 * NKC], FP)
            nc.gpsimd.memset(ones_t[:], 1.0)
            ones_ap = vaug[:].rearrange("p (n c) -> p n c", c=65)[:, :, 64:65]
            nc.scalar.activation(
                _r(ones_ap), ones_t[:].rearrange("p (n c) -> p n c", c=1),
                AF.Copy, bias=0.0, scale=1.0,
            )
            for h in range(2):
                for kc in range(NKC):
                    tp = vtpool.tile([128, 64], FP, tag="vtp")
                    nc.tensor.transpose(
                        tp[:], vT[h * 64:(h + 1) * 64, kc * 128:(kc + 1) * 128],
                        identity[h * 64:(h + 1) * 64, h * 64:(h + 1) * 64],
                    )
                    col = (h * NKC + kc) * 65
                    nc.scalar.activation(
                        _r(vaug[:, col:col + 64]), tp[:], AF.Copy, bias=0.0, scale=1.0
                    )

    # ================= phase B: attention =================
    ctx_pool = ctx.enter_context(tc.tile_pool(name="ctxp", bufs=1))
    ctxn = ctx_pool.tile([128, NKC * QCOLS], FP)  # natural [2048 q, 512 d]
    with tc.tile_pool(name="expT", bufs=2) as epool, \
         tc.tile_pool(name="stage", bufs=2) as stpool, \
         tc.tile_pool(name="csb", bufs=2) as cspool, \
         tc.tile_pool(name="sps", bufs=3, space="PSUM") as spsum, \
         tc.tile_pool(name="cps", bufs=2, space="PSUM") as cpsum, \
         tc.tile_pool(name="tps", bufs=2, space="PSUM") as tpsum, \
         tc.tile_pool(name="ctps", bufs=1, space="PSUM") as ctpsum:
        for qh in range(LQH):
            h = qh // 4                     # local kv head
            # qT m-tile mq holds local q-heads (mq: rows 0-63, mq+4: rows 64-127)
            # so each head's rows share the partition base of its kv head.
            mq, row0 = qh % 4, h * 64
            for qb in range(NQB):
                q0 = qb * QB
                expT = epool.tile([128, NKC * QB], FP, tag="expT")
                cps = cpsum.tile([65, QB], FP, tag="cps")
                # scores^T + exp
                for kc in range(NKC):
                    sps = spsum.tile([128, QB], FP, tag="sps")
                    nc.tensor.matmul(
                        sps[:],
                        lhsT=_r(kT[h * 64:(h + 1) * 64, kc * 128:(kc + 1) * 128]),
                        rhs=_r(qT[row0:row0 + 64, mq * S + q0: mq * S + q0 + QB]),
                        start=True, stop=True,
                    )
                    nc.scalar.activation(
                        _r(expT[:, kc * QB:(kc + 1) * QB]), sps[:], AF.Exp,
                        bias=0.0, scale=1.0,
                    )
                # ctx^T accumulation (row 64 = softmax denominators)
                for kc in range(NKC):
                    va0 = (h * NKC + kc) * 65
                    nc.tensor.matmul(
                        cps[:],
                        lhsT=_r(vaug[:, va0:va0 + 65]),
                        rhs=_r(expT[:, kc * QB:(kc + 1) * QB]),
                        start=(kc == 0), stop=(kc == NKC - 1),
                    )
                # evac ctx^T -> sbuf, transpose to natural, recip + normalize
                csb = cspool.tile([65, QB], FP, tag="csb")
                nc.vector.tensor_copy(csb[:], cps[:])
                for st in range(QB // 128):
                    qt = qb * 2 + st        # global q-tile index 0..15
                    ctp = ctpsum.tile([128, 65], FP, tag="ctp")
                    nc.tensor.transpose(
                        ctp[:], csb[0:65, st * 128:(st + 1) * 128], identity[0:65, 0:65]
                    )
                    rc = recip[:, qh * NKC + qt: qh * NKC + qt + 1]
                    nc.vector.reciprocal(rc, ctp[:, 64:65])
                    nc.vector.tensor_scalar_mul(
                        ctxn[:, qt * QCOLS + qh * 64: qt * QCOLS + qh * 64 + 64],
                        ctp[:, 0:64], rc,
                    )
                # transpose attn tiles back to (q, keys), normalize, stage, store
                stage = stpool.tile([128, 2 * S], FP, tag="stage")
                for st in range(QB // 128):
                    qt = qb * 2 + st
                    rc = recip[:, qh * NKC + qt: qh * NKC + qt + 1]
                    for kg in range(NKC // 4):   # groups of 4 key-chunks
                        tp = tpsum.tile([128, 512], FP, tag="tps")
                        for j in range(4):
                            kc = kg * 4 + j
                            nc.tensor.transpose(
                                tp[:, j * 128:(j + 1) * 128],
                                expT[:, kc * QB + st * 128: kc * QB + st * 128 + 128],
                                identity[:],
                            )
                        nc.vector.tensor_scalar_mul(
                            stage[:, st * S + kg * 512: st * S + (kg + 1) * 512],
                            tp[:], rc,
                        )
                nc.sync.dma_start(
                    attn[qh, q0:q0 + QB, :].rearrange("(c p) k -> p c k", p=128),
                    stage[:].rearrange("p (c k) -> p c k", k=S),
                )

    # ================= phase C: out projection =================
    with tc.tile_pool(name="ctxT", bufs=1) as ctpool, \
         tc.tile_pool(name="wop", bufs=1) as wopool, \
         tc.tile_pool(name="ostage", bufs=2) as ospool, \
         tc.tile_pool(name="ctp2", bufs=2, space="PSUM") as ctp2sum, \
         tc.tile_pool(name="ops", bufs=4, space="PSUM") as opsum:
        ctxnT = ctpool.tile([128, 4 * S], FP)   # 4 d-chunks x tokens
        for t in range(NKC):
            for kd in range(4):
                tp = ctp2sum.tile([128, 128], FP, tag="ctp2")
                nc.tensor.transpose(
                    tp[:], ctxn[:, t * QCOLS + kd * 128: t * QCOLS + (kd + 1) * 128],
                    identity[:],
                )
                nc.scalar.activation(
                    _r(ctxnT[:, kd * S + t * 128: kd * S + t * 128 + 128]),
                    tp[:], AF.Copy, bias=0.0, scale=1.0,
                )
        wo_sb = wopool.tile([128, 4 * E], FP)
        nc.sync.dma_start(
            _r(wo_sb[:].rearrange("p (c n) -> p c n", n=E)),
            _r(wo[:, :].rearrange("(c p) n -> p c n", p=128)),
        )
        for mt in range(NKC):
            ostage = ospool.tile([128, E], FP, tag="ostage")
            for nb in range(4):
                ps = opsum.tile([128, 512], FP, tag="ops")
                for kd in range(4):
                    nc.tensor.matmul(
                        ps[:],
                        lhsT=_r(ctxnT[:, kd * S + mt * 128: kd * S + mt * 128 + 128]),
                        rhs=_r(wo_sb[:, kd * E + nb * 512: kd * E + (nb + 1) * 512]),
                        start=(kd == 0), stop=(kd == 3),
                    )
                nc.scalar.activation(
                    ostage[:, nb * 512:(nb + 1) * 512], ps[:], AF.Copy,
                    bias=0.0, scale=1.0,
                )
            nc.sync.dma_start(outp[mt * 128:(mt + 1) * 128, :], ostage[:])


_NC_CACHE = {}


def _get_nc():
    if "nc" not in _NC_CACHE:
        _NC_CACHE["nc"] = build_nc()
    return _NC_CACHE["nc"]


def kernel(query, key, value, Wq, bq, Wk, bk, Wv, bv, Wo, bo):
    query = np.asarray(query, dtype=np.float32)
    key = np.asarray(key, dtype=np.float32)
    value = np.asarray(value, dtype=np.float32)
    Wq = np.asarray(Wq, dtype=np.float32)
    bq = np.asarray(bq, dtype=np.float32)
    Wk = np.asarray(Wk, dtype=np.float32)
    bk = np.asarray(bk, dtype=np.float32)
    Wv = np.asarray(Wv, dtype=np.float32)
    bv = np.asarray(bv, dtype=np.float32)
    Wo = np.asarray(Wo, dtype=np.float32)
    bo = np.asarray(bo, dtype=np.float32)

    B = query.shape[0]
    scale = np.float32(D ** -0.5)

    xT = {}
    for b in range(B):
        xT[("q", b)] = np.ascontiguousarray(query[b].T)
        xT[("k", b)] = np.ascontiguousarray(key[b].T)
        xT[("v", b)] = np.ascontiguousarray(value[b].T)

    in_maps = []
    for core in range(8):
        b, hp = core // 4, core % 4
        wq_raw = Wq[:, hp * QCOLS:(hp + 1) * QCOLS] * scale
        bq_raw = bq[hp * QCOLS:(hp + 1) * QCOLS] * scale
        # permute local q-heads so m-tile m = [qh=m | qh=m+4] (partition bases
        # 0 / 64 match the kv head each group attends with)
        perm = []
        for m in range(4):
            perm.extend(range(m * 64, (m + 1) * 64))
            perm.extend(range((m + 4) * 64, (m + 5) * 64))
        wq_s = wq_raw[:, perm]
        bq_s = bq_raw[perm].reshape(4, 128).T
        wk_s = np.ascontiguousarray(Wk[:, hp * KVCOLS:(hp + 1) * KVCOLS])
        wv_s = np.ascontiguousarray(Wv[:, hp * KVCOLS:(hp + 1) * KVCOLS])
        bkv_s = np.stack(
            [bk[hp * KVCOLS:(hp + 1) * KVCOLS], bv[hp * KVCOLS:(hp + 1) * KVCOLS]],
            axis=1,
        )
        wo_s = np.ascontiguousarray(Wo[hp * QCOLS:(hp + 1) * QCOLS, :])
        in_maps.append({
            "xqT": xT[("q", b)],
            "xkT": xT[("k", b)],
            "xvT": xT[("v", b)],
            "wq": np.ascontiguousarray(wq_s),
            "wk": wk_s,
            "wv": wv_s,
            "wo": wo_s,
            "bq": np.ascontiguousarray(bq_s),
            "bkv": np.ascontiguousarray(bkv_s),
        })

    nc = _get_nc()
    global _LAST_IN_MAPS
    _LAST_IN_MAPS = in_maps
    res = run_bass_kernel_spmd(nc, in_maps, list(range(8))).results

    NQH = 32
    out = np.empty((B, S, E), dtype=np.float32)
    attn_full = np.empty((B, NQH, S, S), dtype=np.float32)
    for b in range(B):
        acc = None
        for hp in range(4):
            r = res[b * 4 + hp]
            acc = r["outp"] if acc is None else acc + r["outp"]
            attn_full[b, hp * LQH:(hp + 1) * LQH] = r["attn"]
        out[b] = acc + bo
    return out, attn_full
